# revision 10
# baseline (speedup 1.0000x reference)
"""Trainium2 Bass kernel for nn_AMB3RStage2V4 (scatter_memory).

Pipeline (per reference.py):
  1. voxel hash-table query (gather with null-token fallback)
  2. cross-attention fusion  (X_vggt x voxel feats)
  3. MLP tokenizer (Linear-GELU-LayerNorm-Linear)
  4. voxel scatter-update (confidence-weighted per-voxel mean + EMA)

Sharding: data-parallel over BT (2 frames/core on 8 cores); voxel update
sharded by unique-voxel range with an AllGather of per-point update rows.

Host side does: weight folding (linear algebra on weights only), integer
routing metadata (searchsorted / unique / sort -> gather index tensors),
input layout prep (transpose X to feature-major bf16), and output
unshard/assembly.  All tensor math over data runs on device.
"""
import numpy as np
import ml_dtypes

# ---- problem constants (hardcoded; kernel.py must be self-contained) ----
BT, Np, CV = 16, 1369, 2048
HID, HEADS, MEM = 256, 4, 128
Dh = HID // HEADS          # 64
K = 16384                  # voxel store size
GRID = 64
VOXEL_SIZE = 0.05
ALPHA = 0.5
N = BT * Np                # 21904
NCORES = 8
FPC = BT // NCORES         # frames per core = 2
TPC = FPC * Np             # tokens per core = 2738
H1 = 512                   # tokenizer hidden
M2O = MEM + 1              # 129
LN_EPS = 1e-5
DEN_EPS = 1e-8
BF16 = ml_dtypes.bfloat16

P = 128


def _frame_tiles():
    """Token tiles within one frame: 10 full 128-tiles + one 89 tile."""
    out = []
    off = 0
    while off < Np:
        sz = min(P, Np - off)
        out.append((off, sz))
        off += sz
    return out


def _qchunks():
    """Free-dim chunks of one frame's tokens for N<=512 matmuls."""
    out = []
    off = 0
    while off < Np:
        sz = min(512, Np - off)
        out.append((off, sz))
        off += sz
    return out


def _slot_tiles():
    """Tiles over this core's 2738 update output slots."""
    out = []
    off = 0
    while off < TPC:
        sz = min(P, TPC - off)
        out.append((off, sz))
        off += sz
    return out


def _build(R):
    """Build the per-core SPMD Bass graph. R = number of gather levels for
    the per-voxel segment sum."""
    import concourse.bass as bass
    import concourse.bacc as bacc
    import concourse.mybir as mybir
    import concourse.tile as tile
    from concourse.masks import make_identity

    dt = mybir.dt
    AF = mybir.ActivationFunctionType
    ALU = mybir.AluOpType

    nc = bacc.Bacc(None, target_bir_lowering=False)

    # ---------------- I/O ----------------
    xt_p = nc.declare_dram_parameter("xt", [CV, TPC], dt.bfloat16, isOutput=False)
    wq_p = nc.declare_dram_parameter("wq", [CV, HID], dt.bfloat16, isOutput=False)
    wk_p = nc.declare_dram_parameter("wk", [MEM, HID], dt.bfloat16, isOutput=False)
    wv_p = nc.declare_dram_parameter("wv", [MEM, HID], dt.bfloat16, isOutput=False)
    wof_p = nc.declare_dram_parameter("wof", [HID, CV], dt.bfloat16, isOutput=False)
    m1_p = nc.declare_dram_parameter("m1", [CV, H1], dt.bfloat16, isOutput=False)
    m2_p = nc.declare_dram_parameter("m2", [H1, M2O], dt.bfloat16, isOutput=False)
    lng_p = nc.declare_dram_parameter("lng", [P, H1 // P], dt.float32, isOutput=False)
    lnb_p = nc.declare_dram_parameter("lnb", [P, H1 // P], dt.float32, isOutput=False)
    tab_p = nc.declare_dram_parameter("tab", [K + 2, MEM], dt.float32, isOutput=False)
    qidx_p = nc.declare_dram_parameter("qidx", [TPC], dt.int32, isOutput=False)
    uidx_p = nc.declare_dram_parameter("uidx", [R * TPC], dt.int32, isOutput=False)
    oidx_p = nc.declare_dram_parameter("oidx", [TPC], dt.int32, isOutput=False)
    scn_p = nc.declare_dram_parameter("scn", [TPC], dt.float32, isOutput=False)

    xf_o = nc.declare_dram_parameter("xf", [CV, TPC], dt.bfloat16, isOutput=True)
    mr_o = nc.declare_dram_parameter("mr", [TPC, MEM], dt.float32, isOutput=True)
    wc_o = nc.declare_dram_parameter("wc", [TPC, 1], dt.float32, isOutput=True)
    mk_o = nc.declare_dram_parameter("mk", [TPC, 1], dt.float32, isOutput=True)
    vx_o = nc.declare_dram_parameter("vx", [TPC, MEM], dt.float32, isOutput=True)

    g_loc = nc.dram_tensor("g_loc", [TPC, M2O], dt.float32)
    g_all = nc.dram_tensor("g_all", [N + 1, M2O], dt.float32, addr_space="Shared")

    FT = _frame_tiles()
    QC = _qchunks()
    ST = _slot_tiles()
    NKC = CV // P      # 16 feature chunks of X
    NH1 = H1 // P      # 4

    with tile.TileContext(nc) as tc:
        with (
            tc.tile_pool(name="const", bufs=1) as const,
            tc.tile_pool(name="xt", bufs=16) as xtp,
            tc.tile_pool(name="stexp", bufs=11) as stp,
            tc.tile_pool(name="qkt", bufs=5) as qkt,
            tc.tile_pool(name="vaug", bufs=1) as vaugp,
            tc.tile_pool(name="h1", bufs=4) as h1p,
            tc.tile_pool(name="h1sq", bufs=4) as h1sqp,
            tc.tile_pool(name="aotok", bufs=11) as aop,
            tc.tile_pool(name="aonorm", bufs=2) as aonp,
            tc.tile_pool(name="lnrow", bufs=5) as lnrp,
            tc.tile_pool(name="lnbc", bufs=2) as lnbcp,
            tc.tile_pool(name="work", bufs=4) as wk,
            tc.tile_pool(name="small", bufs=8) as sm,
            tc.tile_pool(name="psg", bufs=2, space="PSUM") as psg,
            tc.tile_pool(name="psb", bufs=2, space="PSUM") as psb,
        ):
            # ---------------- constants / weights ----------------
            ident = const.tile([P, P], dt.float32)
            make_identity(nc, ident[:])
            ones_bf = const.tile([P, 1], dt.bfloat16)
            nc.vector.memset(ones_bf[:], 1.0)

            wq_sb = const.tile([P, NKC, HID], dt.bfloat16)
            nc.sync.dma_start(out=wq_sb[:], in_=wq_p.rearrange("(i p) m -> p i m", p=P))
            wk_sb = const.tile([P, HID], dt.bfloat16)
            nc.sync.dma_start(out=wk_sb[:], in_=wk_p[:, :])
            wv_sb = const.tile([P, HID], dt.bfloat16)
            nc.sync.dma_start(out=wv_sb[:], in_=wv_p[:, :])
            wof_sb = const.tile([P, HID // P, CV], dt.bfloat16)
            nc.sync.dma_start(out=wof_sb[:], in_=wof_p.rearrange("(i p) m -> p i m", p=P))
            m1_sb = const.tile([P, NKC, H1], dt.bfloat16)
            nc.sync.dma_start(out=m1_sb[:], in_=m1_p.rearrange("(i p) m -> p i m", p=P))
            m2_sb = const.tile([P, NH1, M2O], dt.bfloat16)
            nc.sync.dma_start(out=m2_sb[:], in_=m2_p.rearrange("(i p) m -> p i m", p=P))
            lng_sb = const.tile([P, NH1], dt.float32)
            nc.sync.dma_start(out=lng_sb[:], in_=lng_p[:, :])
            lnb_sb = const.tile([P, NH1], dt.float32)
            nc.sync.dma_start(out=lnb_sb[:], in_=lnb_p[:, :])

            for f in range(FPC):
                fo = f * Np  # token offset of this frame within the core

                # ---------- X_T load (bf16 feature-major) ----------
                xts = []
                for c in range(NKC):
                    t = xtp.tile([P, Np], dt.bfloat16, tag="xt")
                    nc.sync.dma_start(
                        out=t[:], in_=xt_p[c * P:(c + 1) * P, fo:fo + Np]
                    )
                    xts.append(t)

                # ---------- query gather + xm_T + mask ----------
                xm_T = qkt.tile([P, Np], dt.bfloat16, tag="qkt")
                for (off, sz) in FT:
                    qi = sm.tile([P, 1], dt.int32, tag="idx")
                    nc.sync.dma_start(out=qi[:sz], in_=qidx_p[fo + off:fo + off + sz, None])
                    xm = wk.tile([P, MEM], dt.float32, tag="xm")
                    nc.gpsimd.indirect_dma_start(
                        out=xm[:sz], out_offset=None, in_=tab_p[:],
                        in_offset=bass.IndirectOffsetOnAxis(ap=qi[:sz, :1], axis=0),
                    )
                    pt = psg.tile([P, 512], dt.float32, space="PSUM", tag="ps")
                    nc.tensor.transpose(out=pt[:MEM, :sz], in_=xm[:sz, :MEM], identity=ident[:sz, :sz])
                    nc.vector.tensor_copy(out=xm_T[:, off:off + sz], in_=pt[:MEM, :sz])
                    # mask = (qidx != K) as float
                    qif = sm.tile([P, 1], dt.float32, tag="idxf")
                    nc.vector.tensor_copy(out=qif[:sz], in_=qi[:sz])
                    mkf = sm.tile([P, 1], dt.float32, tag="mkf")
                    nc.vector.tensor_scalar(
                        out=mkf[:sz], in0=qif[:sz], scalar1=float(K), scalar2=None,
                        op0=ALU.not_equal,
                    )
                    nc.sync.dma_start(out=mk_o[fo + off:fo + off + sz, :], in_=mkf[:sz])

                # ---------- k_T [2][128, Np], v_aug [128, 11, 261] ----------
                kts = []
                for m in range(HID // P):
                    kt = qkt.tile([P, Np], dt.bfloat16, tag="qkt")
                    for (qo, qn) in QC:
                        pk = psg.tile([P, 512], dt.float32, space="PSUM", tag="ps")
                        nc.tensor.matmul(
                            out=pk[:, :qn], lhsT=wk_sb[:, m * P:(m + 1) * P],
                            rhs=xm_T[:, qo:qo + qn], start=True, stop=True,
                        )
                        nc.vector.tensor_copy(out=kt[:, qo:qo + qn], in_=pk[:, :qn])
                    kts.append(kt)

                v_aug = vaugp.tile([P, len(FT), 261], dt.bfloat16, tag="vaug")
                for ti, (off, sz) in enumerate(FT):
                    pv = psg.tile([P, 512], dt.float32, space="PSUM", tag="ps")
                    nc.tensor.matmul(
                        out=pv[:sz, :HID], lhsT=xm_T[:, off:off + sz],
                        rhs=wv_sb[:, :HID], start=True, stop=True,
                    )
                    dst = v_aug[:sz, ti, :260].rearrange("p (h x) -> p h x", x=65)
                    nc.vector.tensor_copy(
                        out=dst[:, :, :Dh],
                        in_=pv[:sz, :HID].rearrange("p (h d) -> p h d", d=Dh),
                    )
                    nc.vector.memset(dst[:, :, Dh:65], 1.0)

                # ---------- q_T [2][128, Np] ----------
                qts = []
                for m in range(HID // P):
                    qt = qkt.tile([P, Np], dt.bfloat16, tag="qkt")
                    for (qo, qn) in QC:
                        pq = psg.tile([P, 512], dt.float32, space="PSUM", tag="ps")
                        for kc in range(NKC):
                            nc.tensor.matmul(
                                out=pq[:, :qn],
                                lhsT=wq_sb[:, kc, m * P:(m + 1) * P],
                                rhs=xts[kc][:, qo:qo + qn],
                                start=(kc == 0), stop=(kc == NKC - 1),
                            )
                        nc.vector.tensor_copy(out=qt[:, qo:qo + qn], in_=pq[:, :qn])
                    qts.append(qt)

                # ---------- attention ----------
                ao_toks = {}
                for ti, (off, sz) in enumerate(FT):
                    ao_toks[ti] = aop.tile([P, HID], dt.float32, tag="aotok", name=f"aotok_{ti}")
                for h in range(HEADS):
                    kT_h = kts[h // 2][(h % 2) * Dh:(h % 2) * Dh + Dh, :]
                    qT_h = qts[h // 2][(h % 2) * Dh:(h % 2) * Dh + Dh, :]
                    st_es = []
                    for (ko, ksz) in FT:
                        sp = psb.tile([P, Np], dt.float32, space="PSUM", tag="psb")
                        for (qo, qn) in QC:
                            nc.tensor.matmul(
                                out=sp[:ksz, qo:qo + qn],
                                lhsT=kT_h[:, ko:ko + ksz],
                                rhs=qT_h[:, qo:qo + qn],
                                start=True, stop=True,
                            )
                        se = stp.tile([P, Np], dt.bfloat16, tag="stexp")
                        nc.scalar.activation(
                            out=se[:ksz, :], in_=sp[:ksz, :], func=AF.Exp,
                            scale=float(1.0 / np.sqrt(Dh)),
                        )
                        st_es.append(se)
                    for ti, (qo2, qsz) in enumerate(FT):
                        pa = psg.tile([P, 512], dt.float32, space="PSUM", tag="ps")
                        for kt_i, (ko, ksz) in enumerate(FT):
                            nc.tensor.matmul(
                                out=pa[:qsz, :65],
                                lhsT=st_es[kt_i][:ksz, qo2:qo2 + qsz],
                                rhs=v_aug[:ksz, kt_i, 65 * h:65 * h + 65],
                                start=(kt_i == 0), stop=(kt_i == len(FT) - 1),
                            )
                        rec = sm.tile([P, 1], dt.float32, tag="rec")
                        nc.vector.reciprocal(out=rec[:qsz], in_=pa[:qsz, Dh:Dh + 1])
                        nc.scalar.mul(
                            out=ao_toks[ti][:qsz, h * Dh:(h + 1) * Dh],
                            in_=pa[:qsz, :Dh], mul=rec[:qsz, :1],
                        )
                # transpose ao_tok -> ao_norm_T [2][128, Np] bf16
                aonT = []
                for m in range(HID // P):
                    t = aonp.tile([P, Np], dt.bfloat16, tag="aonorm")
                    aonT.append(t)
                for ti, (off, sz) in enumerate(FT):
                    for m in range(HID // P):
                        pt = psg.tile([P, 512], dt.float32, space="PSUM", tag="ps")
                        nc.tensor.transpose(
                            out=pt[:P, :sz], in_=ao_toks[ti][:sz, m * P:(m + 1) * P],
                            identity=ident[:sz, :sz],
                        )
                        nc.vector.tensor_copy(out=aonT[m][:, off:off + sz], in_=pt[:P, :sz])

                # ---------- delta + X_fuse (in place on xts) + output ----------
                for mc in range(NKC):
                    for (qo, qn) in QC:
                        pd = psg.tile([P, 512], dt.float32, space="PSUM", tag="ps")
                        for hc in range(HID // P):
                            nc.tensor.matmul(
                                out=pd[:, :qn],
                                lhsT=wof_sb[:, hc, mc * P:(mc + 1) * P],
                                rhs=aonT[hc][:, qo:qo + qn],
                                start=(hc == 0), stop=(hc == HID // P - 1),
                            )
                        nc.vector.tensor_add(
                            out=xts[mc][:, qo:qo + qn], in0=xts[mc][:, qo:qo + qn],
                            in1=pd[:, :qn],
                        )
                    nc.sync.dma_start(
                        out=xf_o[mc * P:(mc + 1) * P, fo:fo + Np], in_=xts[mc][:]
                    )

                # ---------- mlp1 + gelu (+ square) ----------
                h1ts = []
                for m1c in range(NH1):
                    ht = h1p.tile([P, Np], dt.bfloat16, tag="h1")
                    for (qo, qn) in QC:
                        ph = psg.tile([P, 512], dt.float32, space="PSUM", tag="ps")
                        for kc in range(NKC):
                            nc.tensor.matmul(
                                out=ph[:, :qn],
                                lhsT=m1_sb[:, kc, m1c * P:(m1c + 1) * P],
                                rhs=xts[kc][:, qo:qo + qn],
                                start=(kc == 0), stop=(kc == NKC - 1),
                            )
                        nc.scalar.activation(out=ht[:, qo:qo + qn], in_=ph[:, :qn], func=AF.Gelu)
                    h1ts.append(ht)

                # ---------- LayerNorm over the 512 (partition) axis ----------
                for (qo, qn) in QC:
                    pmu = psg.tile([P, 512], dt.float32, space="PSUM", tag="ps")
                    for m1c in range(NH1):
                        nc.tensor.matmul(
                            out=pmu[:1, :qn], lhsT=ones_bf[:, :1],
                            rhs=h1ts[m1c][:, qo:qo + qn],
                            start=(m1c == 0), stop=(m1c == NH1 - 1),
                        )
                    mu = lnrp.tile([1, 512], dt.float32, tag="lnrow")
                    nc.scalar.mul(out=mu[:1, :qn], in_=pmu[:1, :qn], mul=1.0 / H1)

                    psq = psb.tile([P, Np], dt.float32, space="PSUM", tag="psb")
                    for m1c in range(NH1):
                        sq = h1sqp.tile([P, 512], dt.bfloat16, tag="h1sq")
                        nc.vector.tensor_tensor(
                            out=sq[:, :qn], in0=h1ts[m1c][:, qo:qo + qn],
                            in1=h1ts[m1c][:, qo:qo + qn], op=ALU.mult,
                        )
                        nc.tensor.matmul(
                            out=psq[:1, :qn], lhsT=ones_bf[:, :1], rhs=sq[:, :qn],
                            start=(m1c == 0), stop=(m1c == NH1 - 1),
                        )
                    # var = E[x^2] - mu^2 ; rstd = 1/sqrt(var+eps)
                    msq = lnrp.tile([1, 512], dt.float32, tag="lnrow")
                    nc.scalar.mul(out=msq[:1, :qn], in_=psq[:1, :qn], mul=1.0 / H1)
                    mu2 = lnrp.tile([1, 512], dt.float32, tag="lnrow")
                    nc.vector.tensor_tensor(out=mu2[:1, :qn], in0=mu[:1, :qn], in1=mu[:1, :qn], op=ALU.mult)
                    # var -> msq in place; std -> new tile; rstd -> new tile
                    nc.vector.tensor_tensor(out=msq[:1, :qn], in0=msq[:1, :qn], in1=mu2[:1, :qn], op=ALU.subtract)
                    nc.vector.tensor_scalar_add(out=msq[:1, :qn], in0=msq[:1, :qn], scalar1=LN_EPS)
                    std = lnrp.tile([1, 512], dt.float32, tag="lnrow")
                    nc.scalar.activation(out=std[:1, :qn], in_=msq[:1, :qn], func=AF.Sqrt)
                    rstd = lnrp.tile([1, 512], dt.float32, tag="lnrow")
                    nc.vector.reciprocal(out=rstd[:1, :qn], in_=std[:1, :qn])

                    mub = lnbcp.tile([P, 512], dt.float32, tag="lnbc")
                    nc.gpsimd.partition_broadcast(mub[:, :qn], mu[:1, :qn])
                    rsb = lnbcp.tile([P, 512], dt.float32, tag="lnbc")
                    nc.gpsimd.partition_broadcast(rsb[:, :qn], rstd[:1, :qn])

                    # apply in place: h1 = (h1 - mu) * rstd [* g + b]
                    for m1c in range(NH1):
                        nc.vector.tensor_tensor(
                            out=h1ts[m1c][:, qo:qo + qn], in0=h1ts[m1c][:, qo:qo + qn],
                            in1=mub[:, :qn], op=ALU.subtract,
                        )
                        nc.vector.tensor_tensor(
                            out=h1ts[m1c][:, qo:qo + qn], in0=h1ts[m1c][:, qo:qo + qn],
                            in1=rsb[:, :qn], op=ALU.mult,
                        )
                        nc.scalar.activation(
                            out=h1ts[m1c][:, qo:qo + qn], in_=h1ts[m1c][:, qo:qo + qn],
                            func=AF.Identity, bias=lnb_sb[:, m1c:m1c + 1],
                            scale=lng_sb[:, m1c:m1c + 1],
                        )

                # ---------- mlp2 (token-major out) + outputs + G ----------
                for (off, sz) in FT:
                    po = psg.tile([P, 512], dt.float32, space="PSUM", tag="ps")
                    for m1c in range(NH1):
                        nc.tensor.matmul(
                            out=po[:sz, :M2O],
                            lhsT=h1ts[m1c][:, off:off + sz],
                            rhs=m2_sb[:, m1c, :M2O],
                            start=(m1c == 0), stop=(m1c == NH1 - 1),
                        )
                    mrt = wk.tile([P, MEM], dt.float32, tag="mrt")
                    nc.vector.tensor_copy(out=mrt[:sz], in_=po[:sz, :MEM])
                    nc.sync.dma_start(out=mr_o[fo + off:fo + off + sz, :], in_=mrt[:sz])
                    wct = sm.tile([P, 1], dt.float32, tag="wct")
                    nc.scalar.activation(out=wct[:sz], in_=po[:sz, MEM:M2O], func=AF.Sigmoid)
                    nc.sync.dma_start(out=wc_o[fo + off:fo + off + sz, :], in_=wct[:sz])
                    gt = wk.tile([P, M2O], dt.float32, tag="gt")
                    nc.scalar.mul(out=gt[:sz, :MEM], in_=po[:sz, :MEM], mul=wct[:sz, :1])
                    nc.vector.tensor_copy(out=gt[:sz, MEM:M2O], in_=wct[:sz, :1])
                    nc.sync.dma_start(out=g_loc[fo + off:fo + off + sz, :], in_=gt[:sz, :M2O])

            # ---------------- voxel update ----------------
            zr = sm.tile([1, M2O], dt.float32, tag="zr")
            nc.vector.memset(zr[:], 0.0)
            nc.sync.dma_start(out=g_all[N:N + 1, :], in_=zr[:1, :])

            nc.gpsimd.collective_compute(
                "AllGather", mybir.AluOpType.bypass,
                replica_groups=[list(range(NCORES))],
                ins=[g_loc[:]], outs=[g_all[:N, :]],
            )

            for (so, ssz) in ST:
                acc = wk.tile([P, M2O], dt.float32, tag="acc")
                for r in range(R):
                    ui = sm.tile([P, 1], dt.int32, tag="uidx")
                    nc.sync.dma_start(
                        out=ui[:ssz], in_=uidx_p[r * TPC + so:r * TPC + so + ssz, None]
                    )
                    if r == 0:
                        nc.gpsimd.indirect_dma_start(
                            out=acc[:ssz], out_offset=None, in_=g_all[:],
                            in_offset=bass.IndirectOffsetOnAxis(ap=ui[:ssz, :1], axis=0),
                        )
                    else:
                        gl = wk.tile([P, M2O], dt.float32, tag="gl")
                        nc.gpsimd.indirect_dma_start(
                            out=gl[:ssz], out_offset=None, in_=g_all[:],
                            in_offset=bass.IndirectOffsetOnAxis(ap=ui[:ssz, :1], axis=0),
                        )
                        nc.vector.tensor_add(out=acc[:ssz], in0=acc[:ssz], in1=gl[:ssz])
                den = sm.tile([P, 1], dt.float32, tag="den")
                nc.vector.tensor_scalar_add(out=den[:ssz], in0=acc[:ssz, MEM:M2O], scalar1=DEN_EPS)
                rec = sm.tile([P, 1], dt.float32, tag="recu")
                nc.vector.reciprocal(out=rec[:ssz], in_=den[:ssz])
                vnew = wk.tile([P, MEM], dt.float32, tag="vnew")
                nc.scalar.mul(out=vnew[:ssz], in_=acc[:ssz, :MEM], mul=rec[:ssz, :1])
                oi = sm.tile([P, 1], dt.int32, tag="oidx")
                nc.sync.dma_start(out=oi[:ssz], in_=oidx_p[so:so + ssz, None])
                og = wk.tile([P, MEM], dt.float32, tag="og")
                nc.gpsimd.indirect_dma_start(
                    out=og[:ssz], out_offset=None, in_=tab_p[:],
                    in_offset=bass.IndirectOffsetOnAxis(ap=oi[:ssz, :1], axis=0),
                )
                scn = sm.tile([P, 1], dt.float32, tag="scn")
                nc.sync.dma_start(out=scn[:ssz], in_=scn_p[so:so + ssz, None])
                nc.scalar.mul(out=vnew[:ssz], in_=vnew[:ssz], mul=scn[:ssz, :1])
                nc.scalar.mul(out=og[:ssz], in_=og[:ssz], mul=ALPHA)
                nc.vector.tensor_add(out=vnew[:ssz], in0=vnew[:ssz], in1=og[:ssz])
                nc.sync.dma_start(out=vx_o[so:so + ssz, :], in_=vnew[:ssz])

    nc.finalize()
    return nc


# ------------------------------------------------------------------
# host side
# ------------------------------------------------------------------

def _host_routing(xyz, store_hash):
    v = np.floor(xyz / VOXEL_SIZE).astype(np.int32)
    h = (v[:, 0] * GRID + v[:, 1]) * GRID + v[:, 2]
    pos = np.clip(np.searchsorted(store_hash, h), 0, K - 1)
    hit = store_hash[pos] == h
    qidx = np.where(hit, pos, K).astype(np.int32)

    uh, counts = np.unique(h, return_counts=True)
    U = len(uh)
    R = int(counts.max())
    order = np.argsort(h, kind="stable")
    starts = np.zeros(U + 1, dtype=np.int64)
    np.cumsum(counts, out=starts[1:])
    uidx = np.full((R, N), N, dtype=np.int32)
    for r in range(R):
        sel = counts > r
        uidx[r, np.nonzero(sel)[0]] = order[starts[:-1][sel] + r]
    posu = np.clip(np.searchsorted(store_hash, uh), 0, K - 1)
    hitu = store_hash[posu] == uh
    oidx = np.full(N, K + 1, dtype=np.int32)
    oidx[:U] = np.where(hitu, posu, K + 1)
    sc_new = np.ones(N, dtype=np.float32)
    sc_new[:U] = np.where(hitu, 1.0 - ALPHA, 1.0)
    return qidx, uidx, oidx, sc_new, R


_NC_CACHE = {}


def kernel(X_vggt, xyz, store_hash, store_vals, null_token, Wq_f, Wk_f, Wv_f,
           in_proj_w, in_proj_b, attn_out_w, attn_out_b, fuse_out_w, gamma,
           mlp1_w, mlp1_b, ln_g, ln_b, mlp2_w, mlp2_b):
    from concourse.bass_utils import run_bass_kernel_spmd

    X_vggt = np.asarray(X_vggt, np.float32)
    xyz = np.asarray(xyz, np.float32)
    store_hash = np.asarray(store_hash, np.int32)
    store_vals = np.asarray(store_vals, np.float32)

    # ---- fold weights (host linear algebra on weights only) ----
    ipw = np.asarray(in_proj_w, np.float32)
    Wq_eff = ipw[:HID] @ np.asarray(Wq_f, np.float32)
    Wk_eff = ipw[HID:2 * HID] @ np.asarray(Wk_f, np.float32)
    Wv_eff = ipw[2 * HID:] @ np.asarray(Wv_f, np.float32)
    g = float(np.asarray(gamma).reshape(-1)[0])
    W_of = g * (np.asarray(fuse_out_w, np.float32) @ np.asarray(attn_out_w, np.float32))
    for b in (in_proj_b, attn_out_b, mlp1_b, mlp2_b):
        assert np.all(np.asarray(b) == 0.0), "nonzero bias unsupported by this kernel build"

    qidx, uidx, oidx, sc_new, R = _host_routing(xyz, store_hash)

    table2 = np.concatenate(
        [store_vals, np.asarray(null_token, np.float32).reshape(1, MEM),
         np.zeros((1, MEM), np.float32)], axis=0,
    )

    wq_h = np.ascontiguousarray(Wq_eff.T.astype(BF16))          # [CV, HID]
    wk_h = np.ascontiguousarray(Wk_eff.T.astype(BF16))          # [MEM, HID]
    wv_h = np.ascontiguousarray(Wv_eff.T.astype(BF16))          # [MEM, HID]
    wof_h = np.ascontiguousarray(W_of.T.astype(BF16))           # [HID, CV]
    m1_h = np.ascontiguousarray(np.asarray(mlp1_w, np.float32).T.astype(BF16))  # [CV, H1]
    m2_h = np.ascontiguousarray(np.asarray(mlp2_w, np.float32).T.astype(BF16))  # [H1, M2O]
    nh1 = H1 // P
    lng_h = np.ascontiguousarray(np.asarray(ln_g, np.float32).reshape(nh1, P).T)
    lnb_h = np.ascontiguousarray(np.asarray(ln_b, np.float32).reshape(nh1, P).T)

    in_maps = []
    for c in range(NCORES):
        xs = X_vggt[c * FPC:(c + 1) * FPC].reshape(TPC, CV)
        xt = np.ascontiguousarray(xs.T.astype(BF16))            # [CV, TPC]
        s = slice(c * TPC, (c + 1) * TPC)
        in_maps.append(dict(
            xt=xt, wq=wq_h, wk=wk_h, wv=wv_h, wof=wof_h, m1=m1_h, m2=m2_h,
            lng=lng_h, lnb=lnb_h, tab=table2,
            qidx=np.ascontiguousarray(qidx[s]),
            uidx=np.ascontiguousarray(uidx[:, s].reshape(-1)),
            oidx=np.ascontiguousarray(oidx[s]),
            scn=np.ascontiguousarray(sc_new[s]),
        ))

    if R not in _NC_CACHE:
        _NC_CACHE[R] = _build(R)
    nc = _NC_CACHE[R]

    res = run_bass_kernel_spmd(nc, in_maps, list(range(NCORES)))

    X_fuse = np.empty((BT, Np, CV), np.float32)
    M_readout = np.empty((BT, Np, MEM), np.float32)
    W_conf = np.empty((BT, Np, 1), np.float32)
    mask = np.empty((BT, Np, 1), np.float32)
    vox = np.empty((N, MEM), np.float32)
    for c in range(NCORES):
        r = res.results[c]
        X_fuse[c * FPC:(c + 1) * FPC] = (
            np.asarray(r["xf"], BF16).astype(np.float32).T.reshape(FPC, Np, CV)
        )
        M_readout[c * FPC:(c + 1) * FPC] = r["mr"].reshape(FPC, Np, MEM)
        W_conf[c * FPC:(c + 1) * FPC] = r["wc"].reshape(FPC, Np, 1)
        mask[c * FPC:(c + 1) * FPC] = r["mk"].reshape(FPC, Np, 1)
        vox[c * TPC:(c + 1) * TPC] = r["vx"]

    return X_fuse, M_readout, W_conf, vox, mask
